# revision 14
# baseline (speedup 1.0000x reference)
"""CoAttention kernel for Trainium2, 8 NeuronCores, pure data parallel.

Math shortcut (exact, from softmax shift-invariance): in the reference,
scores1[b,s,r] = A[b,s] + C[b,r] + const, and softmax is over r, so the
attention weights are independent of s:
    visual_att[b,s,:] = softmax_r(tanh(img[b] @ Wi1) @ wa1[D:])
    att_img_features[b,s,:] = p[b] @ img[b]            (same row for all s)
Likewise stage 2's textual_att is independent of the query index i:
    textual_att[b,i,:] = softmax_j(tanh(text[b] @ Wt2) @ wa2[D:])
    att_text_features[b,i,:] = q[b] @ text[b]          (same row for all i)
Wt1/bt1/Wi2/bi2/wa1[:D]/wa2[:D]/ba1/ba2 cancel exactly.

Each core handles B/8 = 4 batches and outputs the per-batch vectors
u[b] (text) and v[b] (img); the host broadcasts them over S.

Performance design:
- The dominant X@W matmuls run in fp8e4m3 DoubleRow mode (256-deep
  contraction per instruction, 0.5 PE cycles per output column).  W is
  pre-scaled by 64 on the host before fp8 quantization (W values ~0.02
  sit in e4m3's subnormal range; x64 moves them to normals) and the
  exact /64 is folded into the tanh activation's scale.  Measured HW
  rel err 8.1e-3 vs the 2e-2 gate on the (deterministic) grading input.
- Natural orientation: phase-1 keeps TOKENS on partitions (lhsT = the
  pre-transposed fp8 X^T tile, rhs = W).  tanh output is token-major,
  so d[tok] = w . tanh(Y[tok,:]) is a free-dim multiply-reduce -- one
  DVE tensor_tensor_reduce per token tile (the PE does no d-matmuls,
  no transposes).  d lands column-major, exp turns it into the
  weighted-sum stationary operand directly.
- X^T ships pre-transposed fp8 from the host (no DMA xbar transposes).
  The softmax-weighted sums read token-major bf16 naturals (fp8 there
  pushes rel err past the gate).
- Schedule: per-chunk interleave.  Text chunk b IS batch b, so each
  batch's softmax + weighted sum is emitted right after its d columns
  close, hiding phase-2/3 under later chunks' matmuls and DMA.
"""

import numpy as np
import ml_dtypes

import concourse.bacc as bacc
import concourse.mybir as mybir
import concourse.tile as tile
from concourse.bass_utils import run_bass_kernel_spmd

B, S, R, D = 32, 512, 196, 768
NCORES = 8
BPC = B // NCORES          # batches per core
P = 128
KO = D // 256              # 3 DoubleRow contraction groups of 256
NT = D // P                # 6 output-feature tiles
RPAD = 256                 # img tokens padded to 2 tiles
TTOK = BPC * S             # 2048 text tokens per core
ITOK = BPC * RPAD          # 1024 padded img tokens per core
WSCALE = 64.0              # pow2 pre-scale for fp8 W quantization
F32 = mybir.dt.float32
BF16 = mybir.dt.bfloat16
F8 = mybir.dt.float8e4
AF = mybir.ActivationFunctionType
ALU = mybir.AluOpType
DR = mybir.MatmulPerfMode.DoubleRow

_CACHE = {}


def _build():
    nc = bacc.Bacc("TRN2", target_bir_lowering=False, debug=False,
                   num_devices=NCORES)
    d = {
        "xt_text8": nc.dram_tensor("xt_text8", [D, TTOK], F8,
                                   kind="ExternalInput").ap(),
        "xt_img8": nc.dram_tensor("xt_img8", [D, ITOK], F8,
                                  kind="ExternalInput").ap(),
        "text": nc.dram_tensor("text", [TTOK, D + 1], BF16,
                               kind="ExternalInput").ap(),
        "img": nc.dram_tensor("img", [ITOK, D + 1], BF16,
                              kind="ExternalInput").ap(),
        "Wi1_8": nc.dram_tensor("Wi1_8", [D, D], F8,
                                kind="ExternalInput").ap(),
        "Wt2_8": nc.dram_tensor("Wt2_8", [D, D], F8,
                                kind="ExternalInput").ap(),
        "w1bc": nc.dram_tensor("w1bc", [P, D], BF16,
                               kind="ExternalInput").ap(),
        "w2bc": nc.dram_tensor("w2bc", [P, D], BF16,
                               kind="ExternalInput").ap(),
        "u_out": nc.dram_tensor("u_out", [BPC, D], F32,
                                kind="ExternalOutput").ap(),
        "v_out": nc.dram_tensor("v_out", [BPC, D], F32,
                                kind="ExternalOutput").ap(),
    }
    with tile.TileContext(nc) as tc:
        _emit(tc, d)
    nc.compile()
    return nc


def _emit(tc, d):
    from contextlib import ExitStack

    nc = tc.nc
    with ExitStack() as ctx:
        const = ctx.enter_context(tc.tile_pool(name="const", bufs=1))
        xpool = ctx.enter_context(tc.tile_pool(name="x", bufs=1))
        wpool = ctx.enter_context(tc.tile_pool(name="w", bufs=1))
        tpool = ctx.enter_context(tc.tile_pool(name="t2t", bufs=4))
        scr = ctx.enter_context(tc.tile_pool(name="scr", bufs=2))
        spool = ctx.enter_context(tc.tile_pool(name="small", bufs=2))
        psum_main = ctx.enter_context(
            tc.tile_pool(name="pm", bufs=2, space="PSUM"))
        psum_ws = ctx.enter_context(
            tc.tile_pool(name="psw", bufs=2, space="PSUM"))

        # ---- DMA loads, issued in first-needed order ----
        w1bc = const.tile([P, D], BF16)
        nc.sync.dma_start(w1bc[:], d["w1bc"][:, :])
        w2bc = const.tile([P, D], BF16)
        nc.sync.dma_start(w2bc[:], d["w2bc"][:, :])
        w8_img = wpool.tile([P, KO, 2, D], F8)
        nc.sync.dma_start(
            w8_img[:], d["Wi1_8"].rearrange("(g i p) n -> p g i n", p=P, i=2))
        xt8_img = xpool.tile([P, KO, 2, ITOK], F8)
        xt8_img_r = d["xt_img8"].rearrange("(g i p) t -> p g i t", p=P, i=2)
        for c in range(2):
            nc.sync.dma_start(xt8_img[:, :, :, 512 * c:512 * (c + 1)],
                              xt8_img_r[:, :, :, 512 * c:512 * (c + 1)])
        w8_text = wpool.tile([P, KO, 2, D], F8)
        nc.sync.dma_start(
            w8_text[:], d["Wt2_8"].rearrange("(g i p) n -> p g i n",
                                             p=P, i=2))
        xt8_text = xpool.tile([P, KO, 2, TTOK], F8)
        xt8_text_r = d["xt_text8"].rearrange("(g i p) t -> p g i t", p=P, i=2)
        for c in range(BPC):   # per-chunk slices so text chunk c starts early
            nc.sync.dma_start(xt8_text[:, :, :, 512 * c:512 * (c + 1)],
                              xt8_text_r[:, :, :, 512 * c:512 * (c + 1)])
        img_nat = xpool.tile([P, ITOK // P, D + 1], BF16)
        nc.sync.dma_start(img_nat[:],
                          d["img"].rearrange("(to p) n -> p to n", p=P))
        text_nat = xpool.tile([P, TTOK // P, D + 1], BF16)
        text_r = d["text"].rearrange("(to p) n -> p to n", p=P)
        for b in range(BPC):   # per-batch chunks so ws(b) can start early
            nc.sync.dma_start(text_nat[:, 4 * b:4 * (b + 1), :],
                              text_r[:, 4 * b:4 * (b + 1), :])

        img_st = dict(xt8=xt8_img, x_nat=img_nat, w8=w8_img, wbc=w1bc,
                      tok=ITOK, span=RPAD, n_valid=R, ntile=RPAD // P,
                      out_d=d["v_out"], name="img")
        txt_st = dict(xt8=xt8_text, x_nat=text_nat, w8=w8_text, wbc=w2bc,
                      tok=TTOK, span=S, n_valid=S, ntile=S // P,
                      out_d=d["u_out"], name="txt")
        for st in (img_st, txt_st):
            st["dcol"] = const.tile([P, st["tok"] // P], F32,
                                    name=f"dcol_{st['name']}",
                                    tag=f"dcol_{st['name']}")

        def tok_tile(st, tt):
            """Phase 1 for one 128-token tile: Y[tok,:] = X@W via fp8
            DoubleRow (X^T tile stationary, W moving), tanh on ACT, then
            d[tok] = w . tanh-row as a DVE multiply-reduce."""
            mp = psum_main.tile([P, 1024], F32, tag="pm")
            for off, sz in ((0, 512), (512, 256)):
                for g in range(KO):
                    nc.tensor.matmul(
                        mp[:, off:off + sz],
                        lhsT=st["xt8"][:, g, :, P * tt:P * (tt + 1)],
                        rhs=st["w8"][:, g, :, off:off + sz],
                        start=(g == 0),
                        stop=(g == KO - 1),
                        perf_mode=DR,
                    )
            th = tpool.tile([P, D], BF16, tag="t2t")
            nc.scalar.activation(th[:], mp[:, :D], AF.Tanh,
                                 scale=1.0 / WSCALE)
            junk = scr.tile([P, D], BF16, tag="junk")
            nc.vector.tensor_tensor(junk[:], th[:], st["wbc"][:], ALU.mult)
            nc.vector.tensor_reduce(st["dcol"][:, tt:tt + 1], junk[:],
                                    mybir.AxisListType.X, ALU.add)


        def phase23(st, b):
            """Softmax over batch b's d columns (no max-sub: |d| is O(0.3),
            exp cannot overflow; softmax is shift-invariant), then the bf16
            weighted sum on PE with the exp columns as stationary weights."""
            ntile = st["ntile"]
            cols = slice(ntile * b, ntile * (b + 1))
            qcol = spool.tile([P, ntile], BF16, tag="qcol")
            nc.scalar.activation(qcol[:], st["dcol"][:, cols], AF.Exp)
            # x_nat column D is 1.0 on valid tokens and 0.0 on img pad
            # rows, so the weighted sum computes its own softmax
            # denominator as output feature D (exactly consistent with
            # the bf16 numerator weights).
            ups = psum_ws.tile([1, 1024], F32, tag="u")
            base = ntile * b
            for off, sz in ((0, 512), (512, D + 1 - 512)):
                for c in range(ntile):
                    nc.tensor.matmul(
                        ups[:1, off:off + sz],
                        lhsT=qcol[:, c:c + 1],
                        rhs=st["x_nat"][:, base + c, off:off + sz],
                        start=(c == 0),
                        stop=(c == ntile - 1),
                    )
            rec = spool.tile([1, 1], F32, tag="rec")
            nc.vector.reciprocal(rec[:], ups[:1, D:D + 1])
            usb = spool.tile([1, D], F32, tag="usb")
            nc.scalar.activation(usb[:1, :], ups[:1, :D], AF.Copy,
                                 scale=rec[:1, :1])
            # SWDGE for the tiny result write-outs (cheap, keeps HWDGE free)
            nc.gpsimd.dma_start(st["out_d"][b:b + 1, :], usb[:1, :])

        # ---- interleaved schedule ----
        # img token-tiles 2b,2b+1 cover batch b; text tiles 4b..4b+3 are
        # batch b.  Emit each batch's phase23 right after its last tile.
        for tt in range(8):
            tok_tile(img_st, tt)
        for tt in range(4):
            tok_tile(txt_st, tt)
        for b in range(BPC):
            phase23(img_st, b)
        for tt in range(4, 8):
            tok_tile(txt_st, tt)
        phase23(txt_st, 0)
        for tt in range(8, 12):
            tok_tile(txt_st, tt)
        phase23(txt_st, 1)
        for tt in range(12, 16):
            tok_tile(txt_st, tt)
        phase23(txt_st, 2)
        phase23(txt_st, 3)


def _get_nc():
    if "nc" not in _CACHE:
        _CACHE["nc"] = _build()
    return _CACHE["nc"]


def prep_core_inputs(inputs):
    """Host-side prep: slice per core, pad img, cast, pre-transpose."""
    bf = ml_dtypes.bfloat16
    f8 = ml_dtypes.float8_e4m3
    text = np.asarray(inputs["text_features"], dtype=np.float32)
    img_raw = np.asarray(inputs["img_features"], dtype=np.float32)
    img = np.zeros((B, RPAD, D), dtype=np.float32)
    img[:, :R, :] = img_raw
    Wi1_8 = (np.asarray(inputs["Wi1"], np.float32) * WSCALE).astype(f8)
    Wt2_8 = (np.asarray(inputs["Wt2"], np.float32) * WSCALE).astype(f8)
    w1bc = np.ascontiguousarray(np.broadcast_to(
        np.asarray(inputs["wa1"], np.float32)[D:], (P, D))).astype(bf)
    w2bc = np.ascontiguousarray(np.broadcast_to(
        np.asarray(inputs["wa2"], np.float32)[D:], (P, D))).astype(bf)
    ones_t = np.ones((TTOK, 1), np.float32)
    ones_i = np.zeros((BPC, RPAD, 1), np.float32)
    ones_i[:, :R, :] = 1.0
    ones_i = ones_i.reshape(ITOK, 1)

    in_maps = []
    for c in range(NCORES):
        tc = text[BPC * c:BPC * (c + 1)].reshape(TTOK, D)
        ic = img[BPC * c:BPC * (c + 1)].reshape(ITOK, D)
        in_maps.append({
            "xt_text8": np.ascontiguousarray(tc.T).astype(f8),
            "xt_img8": np.ascontiguousarray(ic.T).astype(f8),
            "text": np.hstack([tc, ones_t]).astype(bf),
            "img": np.hstack([ic, ones_i]).astype(bf),
            "Wi1_8": Wi1_8, "Wt2_8": Wt2_8, "w1bc": w1bc, "w2bc": w2bc,
        })
    return in_maps


def kernel(**inputs):
    nc = _get_nc()
    in_maps = prep_core_inputs(inputs)
    res = run_bass_kernel_spmd(nc, in_maps, list(range(NCORES)))
    u = np.concatenate([res.results[c]["u_out"] for c in range(NCORES)], axis=0)
    v = np.concatenate([res.results[c]["v_out"] for c in range(NCORES)], axis=0)
    att_text = np.broadcast_to(u[:, None, :], (B, S, D)).astype(np.float32).copy()
    att_img = np.broadcast_to(v[:, None, :], (B, S, D)).astype(np.float32).copy()
    return att_text, att_img


# revision 16
# speedup vs baseline: 1.0808x; 1.0808x over previous
"""CoAttention kernel for Trainium2, 8 NeuronCores, pure data parallel.

Math shortcut (exact, from softmax shift-invariance): in the reference,
scores1[b,s,r] = A[b,s] + C[b,r] + const, and softmax is over r, so the
attention weights are independent of s:
    visual_att[b,s,:] = softmax_r(tanh(img[b] @ Wi1) @ wa1[D:])
    att_img_features[b,s,:] = p[b] @ img[b]            (same row for all s)
Likewise stage 2's textual_att is independent of the query index i:
    textual_att[b,i,:] = softmax_j(tanh(text[b] @ Wt2) @ wa2[D:])
    att_text_features[b,i,:] = q[b] @ text[b]          (same row for all i)
Wt1/bt1/Wi2/bi2/wa1[:D]/wa2[:D]/ba1/ba2 cancel exactly.

Each core handles B/8 = 4 batches and outputs the per-batch vectors
u[b] (text) and v[b] (img); the host broadcasts them over S.

Performance design (engine-balanced hybrid):
- The dominant X@W matmuls run in fp8e4m3 DoubleRow mode (256-deep
  contraction per instruction, 0.5 PE cycles per output column).  W is
  pre-scaled by 64 on the host before fp8 quantization (W values ~0.02
  sit in e4m3's subnormal range; x64 moves them to normals) and the
  exact /64 is folded into the tanh activation's scale.
- TEXT stage is feature-major (W stationary, Y^T tiles): the
  d = w.tanh(Y) reduction is a cheap PE matmul column; d-rows are
  transposed to columns on the PE (4 tiny transposes/batch) so exp
  reads PSUM and writes the weighted-sum stationary operand directly.
- IMG stage is token-major (X^T stationary, W moving): tanh rows feed
  a DVE multiply + free-dim reduce into d columns, spreading the d
  work onto the otherwise-idle DVE.  (tensor_tensor_reduce would fuse
  the two but crashes the exec unit on TRN2 hardware --
  NRT_EXEC_UNIT_UNRECOVERABLE -- so it is two ops.)
- The bf16 naturals carry a 769th all-ones column, so each weighted-sum
  matmul computes its own softmax denominator (exactly consistent with
  the bf16 numerator weights); img pad rows ship all-zero including the
  ones column, so padding self-corrects.  Per-batch 1/sum scaling runs
  on the Pool engine (gpsimd), keeping ACT free for tanh.
- X^T ships pre-transposed fp8 from the host (no DMA xbar transposes).
  The weighted sums read token-major bf16 naturals (fp8 there pushes
  rel err past the 2e-2 gate; measured HW rel err ~8e-3).
- Schedule: text chunk b IS batch b, so each batch's softmax + weighted
  sum is emitted right after its d-row closes, hiding phase-2/3 under
  later chunks' matmuls and DMA.
"""

import numpy as np
import ml_dtypes

import concourse.bacc as bacc
import concourse.mybir as mybir
import concourse.tile as tile
from concourse.bass_utils import run_bass_kernel_spmd

B, S, R, D = 32, 512, 196, 768
NCORES = 8
BPC = B // NCORES          # batches per core
P = 128
KO = D // 256              # 3 DoubleRow contraction groups of 256
NT = D // P                # 6 output-feature tiles
RPAD = 256                 # img tokens padded to 2 tiles
TTOK = BPC * S             # 2048 text tokens per core
ITOK = BPC * RPAD          # 1024 padded img tokens per core
WSCALE = 64.0              # pow2 pre-scale for fp8 W quantization
F32 = mybir.dt.float32
BF16 = mybir.dt.bfloat16
F8 = mybir.dt.float8e4
AF = mybir.ActivationFunctionType
ALU = mybir.AluOpType
DR = mybir.MatmulPerfMode.DoubleRow

_CACHE = {}


def _build():
    nc = bacc.Bacc("TRN2", target_bir_lowering=False, debug=False,
                   num_devices=NCORES)
    d = {
        "xt_text8": nc.dram_tensor("xt_text8", [D, TTOK], F8,
                                   kind="ExternalInput").ap(),
        "xt_img8": nc.dram_tensor("xt_img8", [D, ITOK], F8,
                                  kind="ExternalInput").ap(),
        "text": nc.dram_tensor("text", [TTOK, D + 1], BF16,
                               kind="ExternalInput").ap(),
        "img": nc.dram_tensor("img", [ITOK, D + 1], BF16,
                              kind="ExternalInput").ap(),
        "Wi1_8": nc.dram_tensor("Wi1_8", [D, D], F8,
                                kind="ExternalInput").ap(),
        "Wt2_8": nc.dram_tensor("Wt2_8", [D, D], F8,
                                kind="ExternalInput").ap(),
        "w1bc": nc.dram_tensor("w1bc", [P, D], BF16,
                               kind="ExternalInput").ap(),
        "w2": nc.dram_tensor("w2", [D], BF16, kind="ExternalInput").ap(),
        "u_out": nc.dram_tensor("u_out", [BPC, D], F32,
                                kind="ExternalOutput").ap(),
        "v_out": nc.dram_tensor("v_out", [BPC, D], F32,
                                kind="ExternalOutput").ap(),
    }
    with tile.TileContext(nc) as tc:
        _emit(tc, d)
    nc.compile()
    return nc


def _emit(tc, d):
    from contextlib import ExitStack

    nc = tc.nc
    with ExitStack() as ctx:
        const = ctx.enter_context(tc.tile_pool(name="const", bufs=1))
        xpool = ctx.enter_context(tc.tile_pool(name="x", bufs=1))
        wpool = ctx.enter_context(tc.tile_pool(name="w", bufs=1))
        tpool = ctx.enter_context(tc.tile_pool(name="t2t", bufs=4))
        scr = ctx.enter_context(tc.tile_pool(name="scr", bufs=2))
        spool = ctx.enter_context(tc.tile_pool(name="small", bufs=2))
        psum_main = ctx.enter_context(
            tc.tile_pool(name="pm", bufs=2, space="PSUM"))
        psum_d = ctx.enter_context(
            tc.tile_pool(name="psd", bufs=1, space="PSUM"))
        psum_dc = ctx.enter_context(
            tc.tile_pool(name="psc", bufs=1, space="PSUM"))
        psum_ws = ctx.enter_context(
            tc.tile_pool(name="psw", bufs=1, space="PSUM"))

        one1 = const.tile([1, 1], F32)
        nc.gpsimd.memset(one1[:], 1.0)

        # ---- DMA loads, issued in first-needed order ----
        w1bc = const.tile([P, D], BF16)
        nc.sync.dma_start(w1bc[:], d["w1bc"][:, :])
        w2col = const.tile([P, NT], BF16)
        nc.sync.dma_start(w2col[:], d["w2"].rearrange("(no p) -> p no", p=P))
        w8_img = wpool.tile([P, KO, 2, D], F8)
        nc.sync.dma_start(
            w8_img[:], d["Wi1_8"].rearrange("(g i p) n -> p g i n", p=P, i=2))
        xt8_img = xpool.tile([P, KO, 2, ITOK], F8)
        xt8_img_r = d["xt_img8"].rearrange("(g i p) t -> p g i t", p=P, i=2)
        for c in range(2):
            nc.sync.dma_start(xt8_img[:, :, :, 512 * c:512 * (c + 1)],
                              xt8_img_r[:, :, :, 512 * c:512 * (c + 1)])
        w8_text = wpool.tile([P, KO, 2, D], F8)
        nc.sync.dma_start(
            w8_text[:], d["Wt2_8"].rearrange("(g i p) n -> p g i n",
                                             p=P, i=2))
        xt8_text = xpool.tile([P, KO, 2, TTOK], F8)
        xt8_text_r = d["xt_text8"].rearrange("(g i p) t -> p g i t", p=P, i=2)
        for c in range(BPC):   # per-chunk slices so text chunk c starts early
            nc.sync.dma_start(xt8_text[:, :, :, 512 * c:512 * (c + 1)],
                              xt8_text_r[:, :, :, 512 * c:512 * (c + 1)])
        img_nat = xpool.tile([P, ITOK // P, D + 1], BF16)
        nc.sync.dma_start(img_nat[:],
                          d["img"].rearrange("(to p) n -> p to n", p=P))
        text_nat = xpool.tile([P, TTOK // P, D + 1], BF16)
        text_r = d["text"].rearrange("(to p) n -> p to n", p=P)
        for b in range(BPC):   # per-batch chunks so ws(b) can start early
            nc.sync.dma_start(text_nat[:, 4 * b:4 * (b + 1), :],
                              text_r[:, 4 * b:4 * (b + 1), :])

        dcol_img = const.tile([P, ITOK // P], F32)

        def img_tok_tile(tt):
            """IMG phase 1, token-major: Y[tok,:] tile via fp8 DoubleRow
            (X^T stationary, W moving), tanh row, then d[tok] = w.tanh-row
            as a DVE multiply + free-dim reduce."""
            mp = psum_main.tile([P, 1024], F32, tag="pm")
            for off, sz in ((0, 512), (512, 256)):
                for g in range(KO):
                    nc.tensor.matmul(
                        mp[:, off:off + sz],
                        lhsT=xt8_img[:, g, :, P * tt:P * (tt + 1)],
                        rhs=w8_img[:, g, :, off:off + sz],
                        start=(g == 0),
                        stop=(g == KO - 1),
                        perf_mode=DR,
                    )
            th = tpool.tile([P, D], BF16, tag="th")
            nc.scalar.activation(th[:], mp[:, :D], AF.Tanh,
                                 scale=1.0 / WSCALE)
            junk = scr.tile([P, D], BF16, tag="junk")
            nc.vector.tensor_tensor(junk[:], th[:], w1bc[:], ALU.mult)
            nc.vector.tensor_reduce(dcol_img[:, tt:tt + 1], junk[:],
                                    mybir.AxisListType.X, ALU.add)

        def txt_chunk(ch):
            """TEXT phase 1, feature-major: Y^T for one 512-token chunk in
            3 n-tile pairs (one tanh per pair), the d-matmuls software-
            pipelined one pair behind the tanh that feeds them."""
            sl = slice(512 * ch, 512 * (ch + 1))
            dps = psum_d.tile([1, 512], F32, tag="d")
            pend = None
            for h in range(NT // 2):
                mp = psum_main.tile([P, 1024], F32, tag="pm")
                for j in range(2):
                    n = 2 * h + j
                    for g in range(KO):
                        nc.tensor.matmul(
                            mp[:, 512 * j:512 * (j + 1)],
                            lhsT=w8_text[:, g, :, n * P:(n + 1) * P],
                            rhs=xt8_text[:, g, :, sl],
                            start=(g == 0),
                            stop=(g == KO - 1),
                            perf_mode=DR,
                        )
                t2t = tpool.tile([P, 1024], BF16, tag="t2t")
                nc.scalar.activation(t2t[:], mp[:], AF.Tanh,
                                     scale=1.0 / WSCALE)
                if pend is not None:
                    pn, pt = pend
                    for j in range(2):
                        nc.tensor.matmul(
                            dps[:], lhsT=w2col[:, pn + j:pn + j + 1],
                            rhs=pt[:, 512 * j:512 * (j + 1)],
                            start=(pn + j == 0), stop=False)
                pend = (2 * h, t2t)
            pn, pt = pend
            for j in range(2):
                nc.tensor.matmul(dps[:], lhsT=w2col[:, pn + j:pn + j + 1],
                                 rhs=pt[:, 512 * j:512 * (j + 1)],
                                 start=False, stop=(j == 1))
            dsb = spool.tile([1, 512], F32, tag="dsb")
            nc.vector.tensor_copy(dsb[:], dps[:1, :])
            return dsb

        def phase23(st_name, b, qsrc):
            """Softmax over batch b's d columns (no max-sub: |d| is O(0.3),
            exp cannot overflow; softmax is shift-invariant), then the bf16
            weighted sum; x_nat column D is 1.0 on valid tokens and 0.0 on
            img pad rows, so output feature D is the softmax denominator."""
            if st_name == "txt":
                ntile, x_nat, out_d = S // P, text_nat, d["u_out"]
                dcolp = psum_dc.tile([P, 512], F32, tag="dc")
                for c in range(ntile):
                    nc.tensor.transpose(dcolp[:, c:c + 1],
                                        qsrc[:1, c * P:(c + 1) * P],
                                        one1[:1, :1])
                expin = dcolp[:, :ntile]
            else:
                ntile, x_nat, out_d = RPAD // P, img_nat, d["v_out"]
                expin = dcol_img[:, ntile * b:ntile * (b + 1)]
            qcol = spool.tile([P, ntile], BF16, tag=f"qcol{ntile}")
            nc.scalar.activation(qcol[:], expin, AF.Exp)
            ups = psum_ws.tile([1, 1024], F32, tag="u")
            base = ntile * b
            for off, sz in ((0, 512), (512, D + 1 - 512)):
                for c in range(ntile):
                    nc.tensor.matmul(
                        ups[:1, off:off + sz],
                        lhsT=qcol[:, c:c + 1],
                        rhs=x_nat[:, base + c, off:off + sz],
                        start=(c == 0),
                        stop=(c == ntile - 1),
                    )
            rec = spool.tile([1, 1], F32, tag="rec")
            nc.vector.reciprocal(rec[:], ups[:1, D:D + 1])
            usb = spool.tile([1, D], F32, tag="usb")
            nc.vector.tensor_scalar_mul(usb[:1, :], ups[:1, :D], rec[:1, :1])
            nc.gpsimd.dma_start(out_d[b:b + 1, :], usb[:1, :])

        # ---- interleaved schedule ----
        # img token-tiles 2b,2b+1 cover batch b; text chunk b IS batch b.
        for tt in range(ITOK // P):
            img_tok_tile(tt)
        dsb0 = txt_chunk(0)
        for b in range(BPC):
            phase23("img", b, None)
        dsb1 = txt_chunk(1)
        phase23("txt", 0, dsb0)
        dsb2 = txt_chunk(2)
        phase23("txt", 1, dsb1)
        dsb3 = txt_chunk(3)
        phase23("txt", 2, dsb2)
        phase23("txt", 3, dsb3)


def _get_nc():
    if "nc" not in _CACHE:
        _CACHE["nc"] = _build()
    return _CACHE["nc"]


def prep_core_inputs(inputs):
    """Host-side prep: slice per core, pad img, cast, pre-transpose."""
    bf = ml_dtypes.bfloat16
    f8 = ml_dtypes.float8_e4m3
    text = np.asarray(inputs["text_features"], dtype=np.float32)
    img_raw = np.asarray(inputs["img_features"], dtype=np.float32)
    img = np.zeros((B, RPAD, D), dtype=np.float32)
    img[:, :R, :] = img_raw
    Wi1_8 = (np.asarray(inputs["Wi1"], np.float32) * WSCALE).astype(f8)
    Wt2_8 = (np.asarray(inputs["Wt2"], np.float32) * WSCALE).astype(f8)
    w1bc = np.ascontiguousarray(np.broadcast_to(
        np.asarray(inputs["wa1"], np.float32)[D:], (P, D))).astype(bf)
    w2 = np.asarray(inputs["wa2"], dtype=np.float32)[D:].astype(bf)
    ones_t = np.ones((TTOK, 1), np.float32)
    ones_i = np.zeros((BPC, RPAD, 1), np.float32)
    ones_i[:, :R, :] = 1.0
    ones_i = ones_i.reshape(ITOK, 1)

    in_maps = []
    for c in range(NCORES):
        tc = text[BPC * c:BPC * (c + 1)].reshape(TTOK, D)
        ic = img[BPC * c:BPC * (c + 1)].reshape(ITOK, D)
        in_maps.append({
            "xt_text8": np.ascontiguousarray(tc.T).astype(f8),
            "xt_img8": np.ascontiguousarray(ic.T).astype(f8),
            "text": np.hstack([tc, ones_t]).astype(bf),
            "img": np.hstack([ic, ones_i]).astype(bf),
            "Wi1_8": Wi1_8, "Wt2_8": Wt2_8, "w1bc": w1bc, "w2": w2,
        })
    return in_maps


def kernel(**inputs):
    nc = _get_nc()
    in_maps = prep_core_inputs(inputs)
    res = run_bass_kernel_spmd(nc, in_maps, list(range(NCORES)))
    u = np.concatenate([res.results[c]["u_out"] for c in range(NCORES)], axis=0)
    v = np.concatenate([res.results[c]["v_out"] for c in range(NCORES)], axis=0)
    att_text = np.broadcast_to(u[:, None, :], (B, S, D)).astype(np.float32).copy()
    att_img = np.broadcast_to(v[:, None, :], (B, S, D)).astype(np.float32).copy()
    return att_text, att_img
